# revision 20
# baseline (speedup 1.0000x reference)
"""Trainium2 Bass kernel for nn_GaussianSplattingDecoder.

Splat 2048 gaussians onto a 200x200x16 voxel grid (V=640000), then a tiny
per-voxel MLP.  Exploits the radius-3 interaction mask: gaussian means are
~N(0,1) while the grid spans +-40 in x/y, so only ~3% of voxel tiles
interact with any gaussian at all.

Strategy (8 NeuronCores, SPMD — one program, per-core data):
  - Voxel tiles of TW=160 contiguous voxels (10 y x 16 z at a single x).
    Host finds, per tile, the candidate gaussians (dist(mean, bbox) < 3),
    packs them into blocks of 128 with tile-centered quadratic-form
    coefficients so both
      A = 0.5*mahalanobis - ln(opacity)   and   B = squared distance
    are K=8 matmuls against shared per-voxel feature rows.
  - Both matmuls run single-pass float32r (the PE truncates operands to
    ~11 mantissa bits but runs at bf16 speed, 4x faster than fp32's
    LOW_HIGH two-pass mode).  Because x is constant per tile, only 5
    feature rows are needed (y'^2/z'^2 via exact small integer-grid
    features, y', z', 1); the 3 spare rows carry hi/lo-compensated
    coefficients: hi parts are pre-rounded to 9 mantissa bits (exact
    pass-through at the hardware's operand truncation, whatever its
    rounding mode) and the lo residuals ride duplicated feature rows.
    Net effective precision ~fp32 — critical for the mask compare
    (B < 9), where ~1e-2 errors flip borderline voxels and flipped
    large-scale gaussians produce O(1) occupancy errors.
  - Device, per (tile, block) unit:  w = exp(-A) * (B < 9);  then
    psum2[18, TW] += semT.T @ w  (semantics cols 0..16, col 17 = 1 -> ws).
    Units are processed in triples sharing one PSUM bank ([128, 480]) so
    exp/STT run as 3x-wide instructions (fixed per-op overhead dominates).
  - Per-tile epilogue, batched over pairs of slots ([18, 320] psum
    accumulators) and engineered off the Scalar engine: DVE computes
    r = 1/max(ws,1e-6); PE broadcasts r; DVE normalizes; MLP layer 1
    takes b1 via the ~1-valued ws row of occ (w1t row 0 = b1); DVE
    applies relu; b2 rides the scalar psum->sbuf copy.  Output stays
    [17, TW]; the host transposes.
  - All inputs load in a few large up-front DMAs (per-descriptor overhead
    on the sync queue is ~0.7us), chunked so the first units' data lands
    first.
  - Inactive voxels get the constant c0 = W2@relu(b1)+b2, filled
    host-side; active tiles are computed into slot-indexed buffers and
    scattered over the fill on the host.
  - Active tiles are sorted by block count and grouped 8 at a time (one
    per core, slot width = group max) so every core runs the identical
    static schedule; missing tiles become dummy all-zero slots which are
    numerically inert and ignored.
"""

import numpy as np
from ml_dtypes import bfloat16

import concourse.bass as bass
import concourse.bacc as bacc
import concourse.mybir as mybir
from concourse import tile
from concourse.bass_utils import run_bass_kernel_spmd

AF = mybir.ActivationFunctionType
ALU = mybir.AluOpType
F32 = mybir.dt.float32
F32R = mybir.dt.float32r
BF16 = mybir.dt.bfloat16

OCC = (200, 200, 16)
PCR_Y0, PCR_Y1 = -40.0, 40.0
PCR_Z0, PCR_Z1 = -1.0, 5.4
V = OCC[0] * OCC[1] * OCC[2]
C = 17
R2 = 9.0
TW = 160           # voxels per tile
BLK = 128          # gaussians per block
N_CORES = 8
GN = 3             # units per exp/STT group (PSUM bank holds 480 f32)
EB = 2             # slots per batched epilogue
VPC = V // N_CORES


# ----------------------------------------------------------------- host math
def _softplus64(x):
    return np.logaddexp(0.0, x.astype(np.float64))


def _log_sigmoid64(x):
    x = x.astype(np.float64)
    return np.where(x >= 0, -np.log1p(np.exp(-np.abs(x))),
                    x - np.log1p(np.exp(-np.abs(x))))


def _rne9(x):
    """Round to 9 explicit mantissa bits (exactly representable at the
    hardware's ~11-bit float32r operand truncation)."""
    x32 = np.asarray(x, np.float32)
    u = x32.view(np.uint32)
    out = ((u + np.uint32(1 << 13)) & np.uint32(0xFFFFC000)).view(np.float32)
    return np.where(np.isfinite(x32), out, x32).astype(np.float64)


def _plan_and_pack(gaussian_props, voxel_coords):
    """Compute the sparse schedule and per-core packed inputs."""
    gp = np.asarray(gaussian_props, np.float32)[0]          # (N, 28)
    vc = np.asarray(voxel_coords, np.float32)               # (V, 3)
    means = gp[:, :3].astype(np.float64)
    scales = _softplus64(gp[:, 3:6])
    inv_s = 1.0 / np.clip(scales * scales, 1e-6, None)
    logop = _log_sigmoid64(gp[:, 10])
    sem = gp[:, 11:11 + C]

    nt = V // TW
    vt = vc.reshape(nt, TW, 3)
    lo, hi = vt.min(1), vt.max(1)

    # candidate gaussians per tile: dist(mean, bbox) < 3
    m32 = gp[:, :3]
    tiles = []  # (tile_id, idx array)
    for s in range(0, nt, 1024):
        e = min(s + 1024, nt)
        cl = np.clip(m32[None, :, :], lo[s:e, None, :], hi[s:e, None, :])
        d2 = ((cl - m32[None, :, :]) ** 2).sum(-1)
        for i in range(e - s):
            idx = np.nonzero(d2[i] < R2)[0]
            if len(idx):
                tiles.append((s + i, idx))

    # sort tiles by candidate count desc, group 8 at a time (one per
    # core); each group is one slot whose width J is the group's max nb.
    # Run slots in ASCENDING width order: tiny slots start compute while
    # the bulk of the coefficient data is still streaming in, and every
    # epilogue (except the final one) hides behind a bigger next slot.
    tiles.sort(key=lambda t: -len(t[1]))
    groups = [tiles[g:g + N_CORES] for g in range(0, len(tiles), N_CORES)]
    groups.reverse()
    schedule = [(len(grp[0][1]) + BLK - 1) // BLK for grp in groups]
    S = len(schedule)                            # slots per core
    U = sum(schedule)                            # units per core

    featsa = np.zeros((N_CORES, S, 8, TW), np.float32)
    featsb = np.zeros((N_CORES, S, 8, TW), np.float32)
    lhsa = np.zeros((N_CORES, U, 8, BLK), np.float32)
    lhsb = np.zeros((N_CORES, U, 8, BLK), np.float32)
    semt = np.zeros((N_CORES, U, BLK, C + 1), bfloat16)
    # (core, slot) -> tile_id for output scatter; -1 = dummy
    slot_tile = np.full((N_CORES, S), -1, np.int64)

    # tile-local integer grid (tile = 10 y x 16 z at a single x)
    iy = np.arange(TW) // 16
    iz = np.arange(TW) % 16
    Fy = iy - 4.5                      # exact small values
    Fz = iz - 7.5
    dy = (PCR_Y1 - PCR_Y0) / (OCC[1] - 1)
    dz = (PCR_Z1 - PCR_Z0) / (OCC[2] - 1)

    for core in range(N_CORES):
        uid = 0
        for sid, J in enumerate(schedule):
            grp = groups[sid]
            if core < len(grp):
                tid, idx = grp[core]
                slot_tile[core, sid] = tid
                ctr = (0.5 * (lo[tid].astype(np.float64)
                              + hi[tid].astype(np.float64)))
                yv = vt[tid][:, 1].astype(np.float64) - ctr[1]
                zv = vt[tid][:, 2].astype(np.float64) - ctr[2]
                y2z2 = yv * yv + zv * zv
                fq_hi = _rne9(y2z2)
                # features: A uses the exact integer-grid quadratics;
                # B folds y'^2+z'^2 into a hi/lo pair (coeff 1)
                featsa[core, sid] = np.stack([
                    Fy * Fy, Fz * Fz, Fy, Fz, np.ones(TW),
                    Fy, Fz, np.ones(TW)]).astype(np.float32)
                featsb[core, sid] = np.stack([
                    fq_hi, y2z2 - fq_hi, Fy, Fz, np.ones(TW),
                    Fy, Fz, np.ones(TW)]).astype(np.float32)
                m = means[idx] - ctr[None, :]
                iv = inv_s[idx]
                n = len(idx)
                ay = 0.5 * iv[:, 1] * dy * dy
                az = 0.5 * iv[:, 2] * dz * dz
                by = -iv[:, 1] * m[:, 1] * dy
                bz = -iv[:, 2] * m[:, 2] * dz
                a0 = 0.5 * (iv[:, 1] * m[:, 1] ** 2
                            + iv[:, 2] * m[:, 2] ** 2
                            + iv[:, 0] * m[:, 0] ** 2) - logop[idx]
                by_h = _rne9(by)
                bz_h = _rne9(bz)
                a0_h = _rne9(a0)
                cA = np.zeros((8, J * BLK), np.float32)
                cA[:, :n] = np.stack([ay, az, by_h, bz_h, a0_h,
                                      by - by_h, bz - bz_h, a0 - a0_h])
                cA[4, n:] = 1e4     # padding: w = exp(-1e4) = 0
                gy = -2.0 * m[:, 1] * dy
                gz = -2.0 * m[:, 2] * dz
                g0 = (m * m).sum(1)
                gy_h = _rne9(gy)
                gz_h = _rne9(gz)
                g0_h = _rne9(g0)
                cS = np.zeros((8, J * BLK), np.float32)
                cS[:, :n] = np.stack([np.ones(n), np.ones(n), gy_h, gz_h,
                                      g0_h, gy - gy_h, gz - gz_h,
                                      g0 - g0_h])
                cS[4, n:] = 1e9     # padding: mask = 0
                # col 0 = 1 (-> ws at psum partition 0), cols 1.. = sem
                sT = np.zeros((J * BLK, C + 1), np.float32)
                sT[:n, 0] = 1.0
                sT[:n, 1:] = sem[idx]
                for j in range(J):
                    lhsa[core, uid + j] = cA[:, j*BLK:(j+1)*BLK]
                    lhsb[core, uid + j] = cS[:, j*BLK:(j+1)*BLK]
                    semt[core, uid + j] = sT[j*BLK:(j+1)*BLK].astype(bfloat16)
            # dummy slots stay all-zero (w=1 but sem=ws=0 -> out=c0)
            uid += J
    return {
        "schedule": schedule, "S": S, "U": U, "slot_tile": slot_tile,
        "featsa": featsa, "featsb": featsb,
        "lhsa": lhsa, "lhsb": lhsb, "semt": semt,
    }


# ------------------------------------------------------------- bass program
def _build_program(schedule, S, U):
    nc = bacc.Bacc("TRN2", target_bir_lowering=False, debug=False,
                   num_devices=N_CORES)

    def din(name, shape, dt=F32):
        return nc.dram_tensor(name, list(shape), dt, kind="ExternalInput").ap()

    def dout(name, shape):
        return nc.dram_tensor(name, list(shape), F32, kind="ExternalOutput").ap()

    # all bulk inputs are pre-transposed on the host so every DMA run is
    # contiguous per partition (strided descriptors measured ~25 GB/s)
    featsr_d = din("featsr", (8, S * TW), F32R)
    featsf_d = din("featsf", (8, S * TW), F32R)
    lhsa_d = din("lhsa", (8, U * BLK), F32R)
    lhsb_d = din("lhsb", (8, U * BLK), F32R)
    semt_d = din("semt", (BLK, U * (C + 1)), BF16)
    w1te_d = din("w1te", (C + 1, 2 * C), BF16)   # row 0 = b1 (occ ws row ~ 1)
    w2e_d = din("w2e", (2 * C, C), BF16)
    b2_d = din("b2", (C, 1))
    slots_d = dout("slots", (S, C, TW))          # host transposes to (TW, C)

    with tile.TileContext(nc) as tc:
        with (
            tc.tile_pool(name="const", bufs=1) as constp,
            tc.tile_pool(name="wp", bufs=4) as wp,
            tc.tile_pool(name="ep", bufs=3) as ep,
            tc.tile_pool(name="psa", bufs=2, space="PSUM") as psa,
            tc.tile_pool(name="psb", bufs=2, space="PSUM") as psb,
            tc.tile_pool(name="ps2", bufs=2, space="PSUM") as ps2p,
            tc.tile_pool(name="pse", bufs=2, space="PSUM") as psep,
        ):
            # constants + all inputs in a few large DMAs
            w1te_s = constp.tile([C + 1, 2 * C], BF16, tag="w1te")
            nc.sync.dma_start(w1te_s[:], w1te_d[:])
            w2e_s = constp.tile([2 * C, C], BF16, tag="w2e")
            nc.sync.dma_start(w2e_s[:], w2e_d[:])
            b2_s = constp.tile([C, 1], F32, tag="b2")
            nc.sync.dma_start(b2_s[:], b2_d[:])
            ones_s = constp.tile([1, C + 1], BF16, tag="ones")
            nc.vector.memset(ones_s[:], 1.0)

            featsr_s = constp.tile([8, S * TW], F32R, tag="featsr")
            nc.sync.dma_start(featsr_s[:], featsr_d[:])
            featsf_s = constp.tile([8, S * TW], F32R, tag="featsf")
            nc.sync.dma_start(featsf_s[:], featsf_d[:])
            # unit data in two chunks as SEPARATE tiles (a shared tile
            # would make the first matmul depend on the last DMA)
            cut = min(sum(schedule[:6]), U)
            lhsa_t = []
            lhsb_t = []
            semt_t = []
            for ci, (u0, u1) in enumerate(((0, cut), (cut, U))):
                if u0 >= u1:
                    continue
                un = u1 - u0
                la = constp.tile([8, un * BLK], F32R, tag=f"lhsa{ci}")
                nc.sync.dma_start(la[:], lhsa_d[:, u0 * BLK:u1 * BLK])
                lb = constp.tile([8, un * BLK], F32R, tag=f"lhsb{ci}")
                nc.sync.dma_start(lb[:], lhsb_d[:, u0 * BLK:u1 * BLK])
                st = constp.tile([BLK, un * (C + 1)], BF16, tag=f"semt{ci}")
                nc.sync.dma_start(
                    st[:], semt_d[:, u0 * (C + 1):u1 * (C + 1)])
                lhsa_t.append(la)
                lhsb_t.append(lb)
                semt_t.append(st)

            def lhsa_sl(u):
                return (lhsa_t[0][:, bass.ts(u, BLK)] if u < cut
                        else lhsa_t[1][:, bass.ts(u - cut, BLK)])

            def lhsb_sl(u):
                return (lhsb_t[0][:, bass.ts(u, BLK)] if u < cut
                        else lhsb_t[1][:, bass.ts(u - cut, BLK)])

            def semt_sl(u):
                return (semt_t[0][:, bass.ts(u, C + 1)] if u < cut
                        else semt_t[1][:, bass.ts(u - cut, C + 1)])

            # unit sequence: (slot, unit, first-in-slot, last-in-slot)
            units = []
            uid = 0
            for sid, J in enumerate(schedule):
                for j in range(J):
                    units.append((sid, uid + j, j == 0, j == J - 1))
                uid += J

            def epilogue(s0, n_slots, p2):
                # ws is p2 row 0; r broadcast via PE; the normalized ws
                # row (~1) carries b1 through w1te row 0
                W = n_slots * TW
                r_s = ep.tile([1, EB * TW], F32, tag="r")
                nc.vector.tensor_scalar_max(r_s[:, 0:W], p2[0:1, 0:W], 1e-6)
                nc.vector.reciprocal_approx_fast(r_s[:, 0:W], r_s[:, 0:W])
                rc_s = ep.tile([1, EB * TW], BF16, tag="rc")
                nc.vector.tensor_scalar_mul(rc_s[:, 0:W], r_s[:, 0:W], 1.0)
                pr = psep.tile([C + 1, EB * TW], F32, tag="pse")
                nc.tensor.matmul(pr[:, 0:W], ones_s[:], rc_s[:, 0:W],
                                 start=True, stop=True)
                rb_s = ep.tile([C + 1, EB * TW], F32, tag="rb")
                nc.scalar.activation(rb_s[:, 0:W], pr[:, 0:W], AF.Copy)
                occ_s = ep.tile([C + 1, EB * TW], BF16, tag="occ")
                nc.vector.tensor_tensor(occ_s[:, 0:W], p2[:, 0:W],
                                        rb_s[:, 0:W], op=ALU.mult)
                ph = psep.tile([2 * C, EB * TW], F32, tag="pse")
                nc.tensor.matmul(ph[:, 0:W], w1te_s[:], occ_s[:, 0:W],
                                 start=True, stop=True)
                h_s = ep.tile([2 * C, EB * TW], BF16, tag="h")
                nc.vector.tensor_scalar_max(h_s[:, 0:W], ph[:, 0:W], 0.0)
                po = psep.tile([C, EB * TW], F32, tag="pse")
                nc.tensor.matmul(po[:, 0:W], w2e_s[:], h_s[:, 0:W],
                                 start=True, stop=True)
                o_s = ep.tile([C, EB * TW], F32, tag="o")
                nc.scalar.activation(o_s[:, 0:W], po[:, 0:W], AF.Identity,
                                     bias=b2_s[:])
                nc.sync.dma_start(
                    slots_d[s0:s0 + n_slots].transpose([1, 0, 2]),
                    o_s[:, 0:W].rearrange("p (k f) -> p k f", f=TW))

            # main sparse loop: units in GN-wide groups sharing PSUM
            # banks; per-tile accumulators batched over EB slots
            p2_tiles = {}
            for p in range(0, len(units), GN):
                grp = units[p:p + GN]
                gw = len(grp) * TW
                pa2 = psa.tile([BLK, GN * TW], F32, tag="pa")
                pb2 = psb.tile([BLK, GN * TW], F32, tag="pb")
                for k, (sid, u, first, last) in enumerate(grp):
                    nc.tensor.matmul(pa2[:, bass.ts(k, TW)],
                                     lhsa_sl(u),
                                     featsr_s[:, bass.ts(sid, TW)],
                                     start=True, stop=True)
                for k, (sid, u, first, last) in enumerate(grp):
                    nc.tensor.matmul(pb2[:, bass.ts(k, TW)],
                                     lhsb_sl(u),
                                     featsf_s[:, bass.ts(sid, TW)],
                                     start=True, stop=True)
                we_s = wp.tile([BLK, GN * TW], BF16, tag="we")
                nc.scalar.activation(we_s[:, 0:gw], pa2[:, 0:gw],
                                     AF.Exp, scale=-1.0)
                w_s = wp.tile([BLK, GN * TW], BF16, tag="w")
                nc.vector.scalar_tensor_tensor(
                    w_s[:, 0:gw], pb2[:, 0:gw], float(R2), we_s[:, 0:gw],
                    op0=ALU.is_lt, op1=ALU.mult)
                for k, (sid, u, first, last) in enumerate(grp):
                    pair = sid // EB
                    if first and pair not in p2_tiles:
                        p2t = ps2p.tile([C + 1, EB * TW], F32, tag="ps2")
                        p2_tiles[pair] = p2t
                    nc.tensor.matmul(
                        p2_tiles[pair][:, bass.ts(sid % EB, TW)],
                        semt_sl(u),
                        w_s[:, bass.ts(k, TW)],
                        start=first, stop=last, skip_group_check=True)
                    if last and (sid % EB == EB - 1 or sid == S - 1):
                        epilogue(pair * EB, sid % EB + 1,
                                 p2_tiles.pop(pair))
    return nc


# ---------------------------------------------------------------- execution
def _execute(nc, plan, W1, b1, W2, b2, trace=False, **kw):
    w1te = np.zeros((C + 1, 2 * C), np.float32)
    w1te[0] = b1
    w1te[1:] = W1.T
    consts = {
        "w1te": w1te.astype(bfloat16),
        "w2e": np.ascontiguousarray(W2.T).astype(bfloat16),
        "b2": b2.reshape(C, 1).astype(np.float32),
    }
    def t8(a):
        # (N, 8, W) -> contiguous (8, N*W)
        return np.ascontiguousarray(
            a.transpose(1, 0, 2).reshape(a.shape[1], -1))

    in_maps = []
    for core in range(N_CORES):
        m = dict(consts)
        m["featsr"] = t8(plan["featsa"][core])
        m["featsf"] = t8(plan["featsb"][core])
        m["lhsa"] = t8(plan["lhsa"][core])
        m["lhsb"] = t8(plan["lhsb"][core])
        m["semt"] = t8(plan["semt"][core])
        in_maps.append(m)
    if not nc.is_finalized():
        nc.finalize()
    return run_bass_kernel_spmd(nc, in_maps, list(range(N_CORES)),
                                trace=trace, **kw)


def _assemble(plan, results, W1, b1, W2, b2):
    # inactive voxels: occ = 0 -> out = W2 @ relu(b1) + b2
    c0 = (W2 @ np.maximum(b1, 0.0) + b2).astype(np.float32)
    out = np.empty((V, C), np.float32)
    out[:] = c0
    slot_tile = plan["slot_tile"]
    for core in range(N_CORES):
        slots = results[core]["slots"]          # (S, C, TW)
        for sid in range(plan["S"]):
            tid = slot_tile[core, sid]
            if tid >= 0:
                out[tid * TW:(tid + 1) * TW] = slots[sid].T
    return out.reshape(1, OCC[0], OCC[1], OCC[2], C)


def run(inputs, trace=False, **kw):
    """Full pipeline; returns (output, BassKernelResults)."""
    gp = np.asarray(inputs["gaussian_props"], np.float32)
    plan = _plan_and_pack(gp, inputs["voxel_coords"])
    nc = _build_program(plan["schedule"], plan["S"], plan["U"])
    W1 = np.asarray(inputs["W1"], np.float32)
    b1 = np.asarray(inputs["b1"], np.float32)
    W2 = np.asarray(inputs["W2"], np.float32)
    b2 = np.asarray(inputs["b2"], np.float32)
    res = _execute(nc, plan, W1, b1, W2, b2, trace=trace, **kw)
    out = _assemble(plan, res.results, W1, b1, W2, b2)
    return out, res


def kernel(**inputs) -> np.ndarray:
    out, _ = run(inputs)
    return out
